# revision 7
# baseline (speedup 1.0000x reference)
"""DeepSeekMoE Trainium2 kernel (8 NeuronCores, SPMD).

Strategy (v3 — group-aligned pieces, dual DMA queues):
  - Host computes top-2 routing and packs the routed tokens into exact-size
    per-expert groups (sum = T*K = 4096 columns, no capacity padding),
    then appends all T tokens once more as the "shared" group (2048 cols):
    one pairs matrix xp [D, 6144].
  - The 8 shared experts (hidden FS=512 each, averaged) are algebraically
    one big FFN with hidden 8*512=4096; its per-core d_ff slice is exactly
    shared expert c.  So every core sees 9 uniform weight sets
    ([1024,512]/[1024,512]/[512,1024]): routed experts 0..7 (f-slice c)
    + shared expert c as weight-set 8.  alpha/NS (=1/16, exact) is folded
    into the shared w2 on the host.
  - v3: work is cut into PIECES that never cross an expert-group boundary
    (variable width <=512 = one PSUM bank of fp32).  Groups wider than 512
    split evenly; the final routed group donates a ~160-col tail piece so
    the post-last-matmul CAST+DMA chain is short.  This removes the v2
    boundary-split matmuls (~660 extra instructions + 25ns-floor penalties).
  - Device: per piece, ffn13 (w1/w3, 8 dt x 4 ft accumulation) -> silu*mul
    -> mm2 (w2) -> psum->sbuf bf16 copy -> DMA out.  mm2 is software-
    pipelined one piece behind ffn13 so the silu/mul latency of the last
    ft never stalls the PE.
  - No gating on device: host multiplies routed outputs by (1-alpha)*prob
    during the combine, so the PE does nothing but N<=512 matmuls at the
    bf16 roofline (fp8 was measured at 5.5e-2 absmax rel err vs the 2e-2
    budget — not usable).
  - DMAs ride TWO HWDGE rings in exact consumption order: weights on the
    GpSimd ring (engine otherwise idle), activations xp + outputs y on the
    Sync ring.  (The Scalar ring serializes against Scalar's ACTIVATE
    stream — measured regression in v1.)  Rings spin up in parallel, so
    the prologue streams W and XP concurrently.  A warmup burst of garbage
    matmuls bridges the ~9us DMA-dead prologue and un-throttles the HAM
    clock gate before the first real matmul issues.
"""

import contextlib

import numpy as np
import ml_dtypes

import concourse.bacc as bacc
import concourse.tile as tile
import concourse.mybir as mybir
from concourse.bass_utils import run_bass_kernel_spmd

BF16 = ml_dtypes.bfloat16

B, S, D, F, E, NS, K = 2, 1024, 1024, 4096, 8, 8, 2
T = B * S
FS = F // NS            # shared expert hidden = 512 (= per-core routed slice)
FL = F // 8             # per-core f-slice of routed experts = 512
NW = E + 1              # 9 weight sets; wid 8 = shared
NCOL = T * K + T        # 6144 pair columns
PW = 512                # max piece width (one PSUM bank of fp32)
ALPHA = 0.5
N_CORES = 8
TAIL_W = 160            # width of the final small piece

KCFG = {
    "warmup_mms": 48,    # garbage matmuls to warm the HAM clock gate and
                         # bridge the PE until the first input DMAs land
    "py_bufs": 3,
    "stream_last": 2,    # per-dt output DMA for the last N pieces
}

_CACHE = {}         # pieces tuple -> compiled program
LAST_RESULT = None  # BassKernelResults of the most recent run (for profiling)


def _pieces_from_sizes(sizes):
    """PE-ordered piece list [(wid, xoff, w), ...].

    Routed groups (wid 0..7, in packing order) are split into <=512-wide
    pieces that never cross a group boundary; the last routed piece is a
    small TAIL_W tail.  The 4 shared 512-wide pieces (wid 8) lead and
    interleave for DMA slack (they need no weight DMA).
    """
    routed = []
    off = 0
    for e in range(E):
        n = int(sizes[e])
        nsub = max(1, -(-n // PW))
        base, rem = divmod(n, nsub)
        o = off
        for j in range(nsub):
            w = base + (1 if j < rem else 0)
            routed.append((e, o, w))
            o += w
        off += n
    # carve the tail piece from the last routed piece
    wid, o, w = routed[-1]
    if w > TAIL_W + 54:
        routed[-1] = (wid, o, w - TAIL_W)
        routed.append((wid, o + w - TAIL_W, TAIL_W))
    shared = [(E, T * K, 256), (E, T * K + 256, 256)] + [
        (E, T * K + 512 * j, 512) for j in range(1, 4)]
    nr = len(routed)
    a = -(-nr // 3)
    order = (shared[:3] + routed[:a] + shared[3:4]
             + routed[a:2 * a] + shared[4:] + routed[2 * a:])
    assert sum(p[2] for p in order) == NCOL
    return order


def _build_program(pieces):
    bf = mybir.dt.bfloat16
    f32 = mybir.dt.float32
    Act = mybir.ActivationFunctionType

    NP = len(pieces)
    # weight-issue schedule: routed wid issued at position (first_use - 2)
    first_use = {}
    for pos, (wid, _, _) in enumerate(pieces):
        if wid != E and wid not in first_use:
            first_use[wid] = pos
    w_sched = {}
    for wid, fu in sorted(first_use.items(), key=lambda kv: kv[1]):
        w_sched.setdefault(max(0, fu - 2), []).append(wid)

    nc = bacc.Bacc("TRN2", target_bir_lowering=False, debug=False,
                   num_devices=N_CORES)

    # weights arrive host-packed in on-chip tile layout [128, a, free]:
    # each partition's data is contiguous in DRAM -> 8 KB DMA descriptors
    xp = nc.dram_tensor("xp", [D, NCOL], bf, kind="ExternalInput").ap()
    w1 = nc.dram_tensor("w1", [NW, 128, 8, FL], bf, kind="ExternalInput").ap()
    w3 = nc.dram_tensor("w3", [NW, 128, 8, FL], bf, kind="ExternalInput").ap()
    w2 = nc.dram_tensor("w2", [NW, 128, 4, D], bf, kind="ExternalInput").ap()
    y = nc.dram_tensor("y", [128, 8, NCOL], bf, kind="ExternalOutput").ap()

    xp_r = xp.rearrange("(a p) t -> p a t", p=128)
    w1_r = [w1[i] for i in range(NW)]
    w3_r = [w3[i] for i in range(NW)]
    w2_r = [w2[i] for i in range(NW)]

    with tile.TileContext(nc) as tc:
        with contextlib.ExitStack() as ctx:
            const = ctx.enter_context(tc.tile_pool(name="const", bufs=1))
            wst = ctx.enter_context(tc.tile_pool(name="wst", bufs=4))
            acts = ctx.enter_context(tc.tile_pool(name="acts", bufs=4))
            hts = ctx.enter_context(tc.tile_pool(name="hts", bufs=2))
            spool = ctx.enter_context(tc.tile_pool(name="spool", bufs=2))
            outs = ctx.enter_context(tc.tile_pool(name="outs", bufs=2))
            psum = ctx.enter_context(
                tc.tile_pool(name="psum", bufs=2, space="PSUM"))
            psy = ctx.enter_context(
                tc.tile_pool(name="psy", bufs=KCFG["py_bufs"], space="PSUM"))
            psw = ctx.enter_context(
                tc.tile_pool(name="psw", bufs=1, space="PSUM"))

            state = {}

            def load_w(wid):
                if wid == E:   # shared set: const pool, ft-split w1/w3
                    W1 = const.tile([128, 8, FL], bf, name="w1s")
                    W3 = const.tile([128, 8, FL], bf, name="w3s")
                    W2 = const.tile([128, 4, D], bf, name="w2s")
                else:
                    W1 = wst.tile([128, 8, FL], bf, tag="w1", name=f"w1_{wid}")
                    W3 = wst.tile([128, 8, FL], bf, tag="w3", name=f"w3_{wid}")
                    W2 = wst.tile([128, 4, D], bf, tag="w2", name=f"w2_{wid}")
                    nc.sync.dma_start(out=W1, in_=w1_r[wid])
                    nc.sync.dma_start(out=W3, in_=w3_r[wid])
                    nc.sync.dma_start(out=W2, in_=w2_r[wid])
                state[("W", wid)] = (W1, W3, W2)
                return W1, W3, W2

            def load_xp(pos, chunks=1):
                wid, o, w = pieces[pos]
                XP = acts.tile([128, 8, PW], bf, tag="xp", name=f"xp{pos}")
                if chunks == 1:
                    nc.sync.dma_start(out=XP[:, :, 0:w],
                                      in_=xp_r[:, :, o:o + w])
                else:
                    for q in range(chunks):
                        a0, a1 = 8 * q // chunks, 8 * (q + 1) // chunks
                        nc.sync.dma_start(out=XP[:, a0:a1, 0:w],
                                          in_=xp_r[:, a0:a1, o:o + w])
                state[("XP", pos)] = XP

            def ffn13(pos):
                wid, _, w = pieces[pos]
                XP = state[("XP", pos)]
                W1, W3, _ = state[("W", wid)]
                HT = hts.tile([128, 4, PW], bf, tag="ht", name=f"ht{pos}")
                for ft in range(4):
                    fsl = slice(ft * 128, (ft + 1) * 128)
                    p1 = psum.tile([128, PW], f32, tag="p1", name=f"p1_{pos}_{ft}")
                    p3 = psum.tile([128, PW], f32, tag="p3", name=f"p3_{pos}_{ft}")
                    for Wt, ps in ((W1, p1), (W3, p3)):
                        for dt in range(8):
                            nc.tensor.matmul(
                                ps[:, 0:w], Wt[:, dt, fsl],
                                XP[:, dt, 0:w],
                                start=(dt == 0), stop=(dt == 7))
                    sa = spool.tile([128, PW], f32, tag="sa", name=f"sa{pos}_{ft}")
                    nc.scalar.activation(sa[:, 0:w], p1[:, 0:w], Act.Silu)
                    nc.vector.tensor_mul(HT[:, ft, 0:w], sa[:, 0:w], p3[:, 0:w])
                state[("HT", pos)] = HT

            def mm2(pos, stream_out=False):
                wid, o, w = pieces[pos]
                HT = state[("HT", pos)]
                W2t = state[("W", wid)][2]
                yo = outs.tile([128, 8, PW], bf, tag="yo", name=f"yo{pos}")
                for dt in range(8):
                    dsl = slice(dt * 128, (dt + 1) * 128)
                    py = psy.tile([128, PW], f32, tag="py", name=f"py{pos}_{dt}")
                    for ft in range(4):
                        nc.tensor.matmul(
                            py[:, 0:w], W2t[:, ft, dsl],
                            HT[:, ft, 0:w],
                            start=(ft == 0), stop=(ft == 3))
                    nc.vector.tensor_copy(out=yo[:, dt, 0:w], in_=py[:, 0:w])
                    # half-piece streaming: per-dt triggers cost ~600ns of
                    # Sync execution each regardless of width — 8 of them
                    # serialize into a multi-us post-matmul tail.
                    if stream_out and dt == 3:
                        nc.sync.dma_start(out=y[:, 0:4, o:o + w],
                                          in_=yo[:, 0:4, 0:w])
                if stream_out:
                    nc.sync.dma_start(out=y[:, 4:8, o:o + w],
                                      in_=yo[:, 4:8, 0:w])
                else:
                    nc.sync.dma_start(out=y[:, :, o:o + w], in_=yo[:, :, 0:w])

            # ---- HAM warmup: garbage matmuls fill the DMA-dead window --
            # The PE clock gate (HAM) needs a few us of sustained activity
            # to un-throttle from 1.2 to 2.4 GHz.  Data DMAs cannot land
            # before ~9us, so burn that window on matmuls over
            # uninitialized SBUF; by the time real matmuls issue, the PE
            # is already warm.
            nwu = KCFG["warmup_mms"]
            if nwu:
                wub = const.tile([128, 128], bf, name="wub")
                wup = psw.tile([128, 128], f32, tag="wu", name="wup")
                nc.vector.memset(wub, 0.0)
                for i in range(nwu):
                    nc.tensor.matmul(wup, wub, wub, start=True, stop=True)

            # ---- prologue: DMAs in exact consumption order per queue ---
            W1S, W3S, W2S = load_w(E)
            wid0, o0, w0 = pieces[0]
            XP0 = acts.tile([128, 8, PW], bf, tag="xp", name="xp0")
            state[("XP", 0)] = XP0
            for q in range(4):
                nc.sync.dma_start(out=XP0[:, 2 * q:2 * q + 2, 0:w0],
                                  in_=xp_r[:, 2 * q:2 * q + 2, o0:o0 + w0])
            nc.sync.dma_start(out=W1S[:, :, 0:128], in_=w1_r[E][:, :, 0:128])
            nc.sync.dma_start(out=W3S[:, :, 0:128], in_=w3_r[E][:, :, 0:128])
            for ft in range(1, 4):
                fsl = slice(ft * 128, (ft + 1) * 128)
                nc.sync.dma_start(out=W1S[:, :, fsl], in_=w1_r[E][:, :, fsl])
                nc.sync.dma_start(out=W3S[:, :, fsl], in_=w3_r[E][:, :, fsl])
            nc.sync.dma_start(out=W2S, in_=w2_r[E])
            load_xp(1)
            for wid in w_sched.get(0, []):
                load_w(wid)
            load_xp(2)

            # ---- main loop: mm2 lags ffn13 by one piece ----------------
            nstream = KCFG["stream_last"]
            for pos in range(NP):
                if pos >= 1:
                    for wid in w_sched.get(pos, []):
                        load_w(wid)
                    if pos + 2 < NP:
                        load_xp(pos + 2)
                ffn13(pos)
                if pos >= 1:
                    mm2(pos - 1, stream_out=(pos - 1 >= NP - nstream))
            mm2(NP - 1, stream_out=True)

    nc.compile()
    return nc


def kernel(hidden_states, gate_W, w1_e, w3_e, w2_e, w1_s, w3_s, w2_s):
    global LAST_RESULT
    x = np.ascontiguousarray(np.asarray(hidden_states, np.float32).reshape(T, D))

    # ---- host routing (sharding decision) + combine coefficients ----
    gate_W = np.asarray(gate_W, np.float32)
    logits = x @ gate_W.T                       # [T, E]
    m = logits.max(axis=1, keepdims=True)
    p = np.exp(logits - m)
    probs = p / p.sum(axis=1, keepdims=True)
    order = np.argsort(-probs, axis=1, kind="stable")[:, :K]   # [T, K]

    idx = [np.where((order == e).any(axis=1))[0] for e in range(E)]
    sizes = tuple(len(te) for te in idx)
    assert sum(sizes) == T * K

    # ---- build device inputs ----------------------------------------
    xT = np.ascontiguousarray(x.T)              # [D, T] fp32
    xf_bf = xT.astype(BF16)                     # [D, T]
    xp_bf = np.empty((D, NCOL), dtype=BF16)
    off = 0
    for e in range(E):
        n = len(idx[e])
        xp_bf[:, off:off + n] = xf_bf[:, idx[e]]
        off += n
    xp_bf[:, T * K:] = xf_bf                    # shared group: all tokens

    w1_e = np.asarray(w1_e, np.float32)
    w3_e = np.asarray(w3_e, np.float32)
    w2_e = np.asarray(w2_e, np.float32)
    w1_s = np.asarray(w1_s, np.float32)
    w3_s = np.asarray(w3_s, np.float32)
    # fold alpha/NS (an exact power of two) into the shared down-proj
    w2_s = np.asarray(w2_s, np.float32) * (ALPHA / NS)

    pieces = tuple(_pieces_from_sizes(sizes))
    nc = _CACHE.get(pieces)
    if nc is None:
        nc = _CACHE[pieces] = _build_program(pieces)

    def _pack(w, na):
        # [NW, na*128, free] -> tile layout [NW, 128, na, free], contiguous
        nw, dd, fr = w.shape
        return np.ascontiguousarray(
            w.reshape(nw, na, 128, fr).transpose(0, 2, 1, 3)).astype(BF16)

    in_maps = []
    for c in range(N_CORES):
        fsl = slice(c * FL, (c + 1) * FL)
        w1c = np.concatenate(
            [np.ascontiguousarray(w1_e[:, :, fsl]), w1_s[c:c + 1]],
            axis=0)
        w3c = np.concatenate(
            [np.ascontiguousarray(w3_e[:, :, fsl]), w3_s[c:c + 1]],
            axis=0)
        w2c = np.concatenate(
            [np.ascontiguousarray(w2_e[:, fsl, :]), w2_s[c:c + 1]],
            axis=0)
        in_maps.append({
            "xp": xp_bf,
            "w1": _pack(w1c, 8),
            "w3": _pack(w3c, 8),
            "w2": _pack(w2c, 4),
        })

    res = run_bass_kernel_spmd(nc, in_maps, list(range(N_CORES)))
    LAST_RESULT = res

    # ---- host combine (unshard + weighted MoE combine) --------------
    yfull = np.zeros((128, 8, NCOL), np.float32)
    for c in range(N_CORES):
        yfull += res.results[c]["y"].astype(np.float32)
    # [p, a, col] -> [a*128+p, col] = [D, NCOL]
    yfull = np.ascontiguousarray(yfull.transpose(1, 0, 2)).reshape(D, NCOL)

    outT = yfull[:, T * K:].copy()              # shared part (scales folded)
    off = 0
    for e in range(E):
        te = idx[e]
        coef = ((1.0 - ALPHA) * probs[te, e]).astype(np.float32)
        outT[:, te] += yfull[:, off:off + len(te)] * coef[None, :]
        off += len(te)

    return np.ascontiguousarray(outT.T).reshape(B, S, D).astype(np.float32)


# revision 8
# speedup vs baseline: 1.0292x; 1.0292x over previous
"""DeepSeekMoE Trainium2 kernel (8 NeuronCores, SPMD).

Strategy (v3 — group-aligned pieces, dual DMA queues):
  - Host computes top-2 routing and packs the routed tokens into exact-size
    per-expert groups (sum = T*K = 4096 columns, no capacity padding),
    then appends all T tokens once more as the "shared" group (2048 cols):
    one pairs matrix xp [D, 6144].
  - The 8 shared experts (hidden FS=512 each, averaged) are algebraically
    one big FFN with hidden 8*512=4096; its per-core d_ff slice is exactly
    shared expert c.  So every core sees 9 uniform weight sets
    ([1024,512]/[1024,512]/[512,1024]): routed experts 0..7 (f-slice c)
    + shared expert c as weight-set 8.  alpha/NS (=1/16, exact) is folded
    into the shared w2 on the host.
  - v3: work is cut into PIECES that never cross an expert-group boundary
    (variable width <=512 = one PSUM bank of fp32).  Groups wider than 512
    split evenly; the final routed group donates a ~160-col tail piece so
    the post-last-matmul CAST+DMA chain is short.  This removes the v2
    boundary-split matmuls (~660 extra instructions + 25ns-floor penalties).
  - Device: per piece, ffn13 (w1/w3, 8 dt x 4 ft accumulation) -> silu*mul
    -> mm2 (w2) -> psum->sbuf bf16 copy -> DMA out.  mm2 is software-
    pipelined one piece behind ffn13 so the silu/mul latency of the last
    ft never stalls the PE.
  - No gating on device: host multiplies routed outputs by (1-alpha)*prob
    during the combine, so the PE does nothing but N<=512 matmuls at the
    bf16 roofline (fp8 was measured at 5.5e-2 absmax rel err vs the 2e-2
    budget — not usable).
  - DMAs ride TWO HWDGE rings in exact consumption order: weights on the
    GpSimd ring (engine otherwise idle), activations xp + outputs y on the
    Sync ring.  (The Scalar ring serializes against Scalar's ACTIVATE
    stream — measured regression in v1.)  Rings spin up in parallel, so
    the prologue streams W and XP concurrently.  A warmup burst of garbage
    matmuls bridges the ~9us DMA-dead prologue and un-throttles the HAM
    clock gate before the first real matmul issues.
"""

import contextlib

import numpy as np
import ml_dtypes

import concourse.bacc as bacc
import concourse.tile as tile
import concourse.mybir as mybir
from concourse.bass_utils import run_bass_kernel_spmd

BF16 = ml_dtypes.bfloat16

B, S, D, F, E, NS, K = 2, 1024, 1024, 4096, 8, 8, 2
T = B * S
FS = F // NS            # shared expert hidden = 512 (= per-core routed slice)
FL = F // 8             # per-core f-slice of routed experts = 512
NW = E + 1              # 9 weight sets; wid 8 = shared
NCOL = T * K + T        # 6144 pair columns
PW = 512                # max piece width (one PSUM bank of fp32)
ALPHA = 0.5
N_CORES = 8
TAIL_W = 160            # width of the final small piece

KCFG = {
    "warmup_mms": 56,    # garbage matmuls to warm the HAM clock gate and
                         # bridge the PE until the first input DMAs land
    "py_bufs": 3,
    "stream_last": 2,    # per-dt output DMA for the last N pieces
}

_CACHE = {}         # pieces tuple -> compiled program
LAST_RESULT = None  # BassKernelResults of the most recent run (for profiling)


def _pieces_from_sizes(sizes):
    """PE-ordered piece list [(wid, xoff, w), ...].

    Routed groups (wid 0..7, in packing order) are split into <=512-wide
    pieces that never cross a group boundary; the last routed piece is a
    small TAIL_W tail.  The 4 shared 512-wide pieces (wid 8) lead and
    interleave for DMA slack (they need no weight DMA).
    """
    routed = []
    off = 0
    for e in range(E):
        n = int(sizes[e])
        nsub = max(1, -(-n // PW))
        base, rem = divmod(n, nsub)
        o = off
        for j in range(nsub):
            w = base + (1 if j < rem else 0)
            routed.append((e, o, w))
            o += w
        off += n
    # carve the tail piece from the last routed piece
    wid, o, w = routed[-1]
    if w > TAIL_W + 54:
        routed[-1] = (wid, o, w - TAIL_W)
        routed.append((wid, o + w - TAIL_W, TAIL_W))
    shared = [(E, T * K + 512 * j, 512) for j in range(4)]
    nr = len(routed)
    a = -(-nr // 3)
    order = (shared[:2] + routed[:a] + shared[2:3]
             + routed[a:2 * a] + shared[3:] + routed[2 * a:])
    assert sum(p[2] for p in order) == NCOL
    return order


def _build_program(pieces):
    bf = mybir.dt.bfloat16
    f32 = mybir.dt.float32
    Act = mybir.ActivationFunctionType

    NP = len(pieces)
    # weight-issue schedule: routed wid issued at position (first_use - 2)
    first_use = {}
    for pos, (wid, _, _) in enumerate(pieces):
        if wid != E and wid not in first_use:
            first_use[wid] = pos
    w13_sched, w2_sched = {}, {}
    for wid, fu in sorted(first_use.items(), key=lambda kv: kv[1]):
        w13_sched.setdefault(max(0, fu - 2), []).append(wid)
        w2_sched.setdefault(max(0, fu - 1), []).append(wid)

    nc = bacc.Bacc("TRN2", target_bir_lowering=False, debug=False,
                   num_devices=N_CORES)

    # weights arrive host-packed in on-chip tile layout [128, a, free]:
    # each partition's data is contiguous in DRAM -> 8 KB DMA descriptors
    xp = nc.dram_tensor("xp", [D, NCOL], bf, kind="ExternalInput").ap()
    w1 = nc.dram_tensor("w1", [NW, 128, 8, FL], bf, kind="ExternalInput").ap()
    w3 = nc.dram_tensor("w3", [NW, 128, 8, FL], bf, kind="ExternalInput").ap()
    w2 = nc.dram_tensor("w2", [NW, 128, 4, D], bf, kind="ExternalInput").ap()
    y = nc.dram_tensor("y", [128, 8, NCOL], bf, kind="ExternalOutput").ap()

    xp_r = xp.rearrange("(a p) t -> p a t", p=128)
    w1_r = [w1[i] for i in range(NW)]
    w3_r = [w3[i] for i in range(NW)]
    w2_r = [w2[i] for i in range(NW)]

    with tile.TileContext(nc) as tc:
        with contextlib.ExitStack() as ctx:
            const = ctx.enter_context(tc.tile_pool(name="const", bufs=1))
            wst = ctx.enter_context(tc.tile_pool(name="wst", bufs=4))
            acts = ctx.enter_context(tc.tile_pool(name="acts", bufs=4))
            hts = ctx.enter_context(tc.tile_pool(name="hts", bufs=2))
            spool = ctx.enter_context(tc.tile_pool(name="spool", bufs=2))
            outs = ctx.enter_context(tc.tile_pool(name="outs", bufs=2))
            psum = ctx.enter_context(
                tc.tile_pool(name="psum", bufs=2, space="PSUM"))
            psy = ctx.enter_context(
                tc.tile_pool(name="psy", bufs=KCFG["py_bufs"], space="PSUM"))
            psw = ctx.enter_context(
                tc.tile_pool(name="psw", bufs=1, space="PSUM"))

            state = {}

            def load_w13(wid):
                W1 = wst.tile([128, 8, FL], bf, tag="w1", name=f"w1_{wid}")
                W3 = wst.tile([128, 8, FL], bf, tag="w3", name=f"w3_{wid}")
                nc.sync.dma_start(out=W1, in_=w1_r[wid])
                nc.sync.dma_start(out=W3, in_=w3_r[wid])
                state[("W13", wid)] = (W1, W3)

            def load_w2(wid):
                W2 = wst.tile([128, 4, D], bf, tag="w2", name=f"w2_{wid}")
                nc.sync.dma_start(out=W2, in_=w2_r[wid])
                state[("W2", wid)] = W2

            def load_xp(pos, chunks=1):
                wid, o, w = pieces[pos]
                XP = acts.tile([128, 8, PW], bf, tag="xp", name=f"xp{pos}")
                if chunks == 1:
                    nc.sync.dma_start(out=XP[:, :, 0:w],
                                      in_=xp_r[:, :, o:o + w])
                else:
                    for q in range(chunks):
                        a0, a1 = 8 * q // chunks, 8 * (q + 1) // chunks
                        nc.sync.dma_start(out=XP[:, a0:a1, 0:w],
                                          in_=xp_r[:, a0:a1, o:o + w])
                state[("XP", pos)] = XP

            def ffn13(pos):
                wid, _, w = pieces[pos]
                XP = state[("XP", pos)]
                W1, W3 = state[("W13", wid)]
                HT = hts.tile([128, 4, PW], bf, tag="ht", name=f"ht{pos}")
                for ft in range(4):
                    fsl = slice(ft * 128, (ft + 1) * 128)
                    p1 = psum.tile([128, PW], f32, tag="p1", name=f"p1_{pos}_{ft}")
                    p3 = psum.tile([128, PW], f32, tag="p3", name=f"p3_{pos}_{ft}")
                    for Wt, ps in ((W1, p1), (W3, p3)):
                        for dt in range(8):
                            nc.tensor.matmul(
                                ps[:, 0:w], Wt[:, dt, fsl],
                                XP[:, dt, 0:w],
                                start=(dt == 0), stop=(dt == 7))
                    sa = spool.tile([128, PW], f32, tag="sa", name=f"sa{pos}_{ft}")
                    nc.scalar.activation(sa[:, 0:w], p1[:, 0:w], Act.Silu)
                    nc.vector.tensor_mul(HT[:, ft, 0:w], sa[:, 0:w], p3[:, 0:w])
                state[("HT", pos)] = HT

            def mm2(pos, stream_out=False):
                wid, o, w = pieces[pos]
                HT = state[("HT", pos)]
                W2t = state[("W2", wid)]
                yo = outs.tile([128, 8, PW], bf, tag="yo", name=f"yo{pos}")
                for dt in range(8):
                    dsl = slice(dt * 128, (dt + 1) * 128)
                    py = psy.tile([128, PW], f32, tag="py", name=f"py{pos}_{dt}")
                    for ft in range(4):
                        nc.tensor.matmul(
                            py[:, 0:w], W2t[:, ft, dsl],
                            HT[:, ft, 0:w],
                            start=(ft == 0), stop=(ft == 3))
                    nc.vector.tensor_copy(out=yo[:, dt, 0:w], in_=py[:, 0:w])
                    # half-piece streaming: per-dt triggers cost ~600ns of
                    # Sync execution each regardless of width — 8 of them
                    # serialize into a multi-us post-matmul tail.
                    if stream_out and dt == 3:
                        nc.sync.dma_start(out=y[:, 0:4, o:o + w],
                                          in_=yo[:, 0:4, 0:w])
                if stream_out:
                    nc.sync.dma_start(out=y[:, 4:8, o:o + w],
                                      in_=yo[:, 4:8, 0:w])
                else:
                    nc.sync.dma_start(out=y[:, :, o:o + w], in_=yo[:, :, 0:w])

            # ---- HAM warmup: garbage matmuls fill the DMA-dead window --
            # The PE clock gate (HAM) needs a few us of sustained activity
            # to un-throttle from 1.2 to 2.4 GHz.  Data DMAs cannot land
            # before ~9us, so burn that window on matmuls over
            # uninitialized SBUF; by the time real matmuls issue, the PE
            # is already warm.
            nwu = KCFG["warmup_mms"]
            if nwu:
                wub = const.tile([128, 128], bf, name="wub")
                wup = psw.tile([128, 128], f32, tag="wu", name="wup")
                nc.vector.memset(wub, 0.0)
                for i in range(nwu):
                    nc.tensor.matmul(wup, wub, wub, start=True, stop=True)

            # ---- prologue: DMAs in exact consumption order per queue ---
            W1S = const.tile([128, 8, FL], bf, name="w1s")
            W3S = const.tile([128, 8, FL], bf, name="w3s")
            W2S = const.tile([128, 4, D], bf, name="w2s")
            state[("W13", E)] = (W1S, W3S)
            state[("W2", E)] = W2S
            wid0, o0, w0 = pieces[0]
            XP0 = acts.tile([128, 8, PW], bf, tag="xp", name="xp0")
            state[("XP", 0)] = XP0
            nc.sync.dma_start(out=W1S[:, :, 0:128], in_=w1_r[E][:, :, 0:128])
            for q in range(4):
                nc.sync.dma_start(out=XP0[:, 2 * q:2 * q + 2, 0:w0],
                                  in_=xp_r[:, 2 * q:2 * q + 2, o0:o0 + w0])
            nc.sync.dma_start(out=W3S[:, :, 0:128], in_=w3_r[E][:, :, 0:128])
            for ft in range(1, 4):
                fsl = slice(ft * 128, (ft + 1) * 128)
                nc.sync.dma_start(out=W1S[:, :, fsl], in_=w1_r[E][:, :, fsl])
                nc.sync.dma_start(out=W3S[:, :, fsl], in_=w3_r[E][:, :, fsl])
            load_xp(1, chunks=2)
            nc.sync.dma_start(out=W2S, in_=w2_r[E])
            for wid in w13_sched.get(0, []):
                load_w13(wid)
            load_xp(2, chunks=2)

            # ---- main loop: mm2 lags ffn13 by one piece ----------------
            nstream = KCFG["stream_last"]
            for pos in range(NP):
                if pos >= 1:
                    for wid in w2_sched.get(pos, []):
                        load_w2(wid)
                    for wid in w13_sched.get(pos, []):
                        load_w13(wid)
                    if pos + 2 < NP:
                        load_xp(pos + 2)
                ffn13(pos)
                if pos >= 1:
                    mm2(pos - 1, stream_out=(pos - 1 >= NP - nstream))
            mm2(NP - 1, stream_out=True)

    nc.compile()
    return nc


def kernel(hidden_states, gate_W, w1_e, w3_e, w2_e, w1_s, w3_s, w2_s):
    global LAST_RESULT
    x = np.ascontiguousarray(np.asarray(hidden_states, np.float32).reshape(T, D))

    # ---- host routing (sharding decision) + combine coefficients ----
    gate_W = np.asarray(gate_W, np.float32)
    logits = x @ gate_W.T                       # [T, E]
    m = logits.max(axis=1, keepdims=True)
    p = np.exp(logits - m)
    probs = p / p.sum(axis=1, keepdims=True)
    order = np.argsort(-probs, axis=1, kind="stable")[:, :K]   # [T, K]

    idx = [np.where((order == e).any(axis=1))[0] for e in range(E)]
    sizes = tuple(len(te) for te in idx)
    assert sum(sizes) == T * K

    # ---- build device inputs ----------------------------------------
    xT = np.ascontiguousarray(x.T)              # [D, T] fp32
    xf_bf = xT.astype(BF16)                     # [D, T]
    xp_bf = np.empty((D, NCOL), dtype=BF16)
    off = 0
    for e in range(E):
        n = len(idx[e])
        xp_bf[:, off:off + n] = xf_bf[:, idx[e]]
        off += n
    xp_bf[:, T * K:] = xf_bf                    # shared group: all tokens

    w1_e = np.asarray(w1_e, np.float32)
    w3_e = np.asarray(w3_e, np.float32)
    w2_e = np.asarray(w2_e, np.float32)
    w1_s = np.asarray(w1_s, np.float32)
    w3_s = np.asarray(w3_s, np.float32)
    # fold alpha/NS (an exact power of two) into the shared down-proj
    w2_s = np.asarray(w2_s, np.float32) * (ALPHA / NS)

    pieces = tuple(_pieces_from_sizes(sizes))
    nc = _CACHE.get(pieces)
    if nc is None:
        nc = _CACHE[pieces] = _build_program(pieces)

    def _pack(w, na):
        # [NW, na*128, free] -> tile layout [NW, 128, na, free], contiguous
        nw, dd, fr = w.shape
        return np.ascontiguousarray(
            w.reshape(nw, na, 128, fr).transpose(0, 2, 1, 3)).astype(BF16)

    in_maps = []
    for c in range(N_CORES):
        fsl = slice(c * FL, (c + 1) * FL)
        w1c = np.concatenate(
            [np.ascontiguousarray(w1_e[:, :, fsl]), w1_s[c:c + 1]],
            axis=0)
        w3c = np.concatenate(
            [np.ascontiguousarray(w3_e[:, :, fsl]), w3_s[c:c + 1]],
            axis=0)
        w2c = np.concatenate(
            [np.ascontiguousarray(w2_e[:, fsl, :]), w2_s[c:c + 1]],
            axis=0)
        in_maps.append({
            "xp": xp_bf,
            "w1": _pack(w1c, 8),
            "w3": _pack(w3c, 8),
            "w2": _pack(w2c, 4),
        })

    res = run_bass_kernel_spmd(nc, in_maps, list(range(N_CORES)))
    LAST_RESULT = res

    # ---- host combine (unshard + weighted MoE combine) --------------
    yfull = np.zeros((128, 8, NCOL), np.float32)
    for c in range(N_CORES):
        yfull += res.results[c]["y"].astype(np.float32)
    # [p, a, col] -> [a*128+p, col] = [D, NCOL]
    yfull = np.ascontiguousarray(yfull.transpose(1, 0, 2)).reshape(D, NCOL)

    outT = yfull[:, T * K:].copy()              # shared part (scales folded)
    off = 0
    for e in range(E):
        te = idx[e]
        coef = ((1.0 - ALPHA) * probs[te, e]).astype(np.float32)
        outT[:, te] += yfull[:, off:off + len(te)] * coef[None, :]
        off += len(te)

    return np.ascontiguousarray(outT.T).reshape(B, S, D).astype(np.float32)
